# revision 1
# baseline (speedup 1.0000x reference)
"""Trainium2 Bass kernel for nn_FDSM_40295383171690.

Math (verified vs reference in fp64, rel err ~4e-7):
  gating: GN(concat(x,x)) == concat(GN4(x), GN4(x)); fold gamma/beta into the
          1x1 conv -> W', b'; weights = softmax(wg @ GAP(relu(W' xn + b')))
  fft:    out = irfft2( rfft2(x)^2 * Wmix ) + r*x
          Wmix[b] = sum_f weights[b,f] * Wsym[f],  Wsym = ds_w filters with
          columns k2 in {0,64} Hermitian-symmetrized along k1 (this absorbs
          the rfft2(irfft2(.)) Hermitian projection exactly).

Sharding: core k = gating for sample k (all C) + FFT branch for channels
[8k,8k+8) of all samples; the [8,4] gating weights are AllGathered on-chip.

DFTs are dense matmuls: stage1 (contract h, lhsT=x, rhs=[Ch|Sh], fp32r N=256),
stage2 (contract w, lhsT=U/V bf16, rhs=[Cw|-Sw],[-Sw|-Cw]), iDFT-A (contract
k1, lhsT=D fp32r, rhs=[Cih|Sih],[-Sih|Cih] N=256), iDFT-B (contract k2,
lhsT=Z2 bf16, rhs=Gc,Gs). Wmix is a K=(f x 32)-packed matmul with col-tiling.
"""

import numpy as np
import ml_dtypes

import concourse.bass as bass
import concourse.bacc as bacc
import concourse.mybir as mybir
import concourse.tile as tile
from concourse.bass_utils import run_bass_kernel_spmd

dt = mybir.dt
AF = mybir.ActivationFunctionType
ALU = mybir.AluOpType
AX = mybir.AxisListType

B, C, H, W, F = 8, 64, 128, 128, 4
WF = 65
NCORES = 8
CS = C // NCORES
EPS = 1e-5
HW = H * W

_cache = {}
DEBUG = False
N_B = 8
SIM_MODE = False


def _build_constants():
    h = np.arange(H)
    k1 = np.arange(H)
    w = np.arange(W)
    k2 = np.arange(WF)
    Ch = np.cos(2 * np.pi * np.outer(h, k1) / H).astype(np.float32)
    Sh = np.sin(2 * np.pi * np.outer(h, k1) / H).astype(np.float32)
    Cw = np.cos(2 * np.pi * np.outer(w, k2) / W).astype(np.float32)
    Sw = np.sin(2 * np.pi * np.outer(w, k2) / W).astype(np.float32)
    Cih = (np.cos(2 * np.pi * np.outer(k1, h) / H) / H).astype(np.float32)
    Sih = (np.sin(2 * np.pi * np.outer(k1, h) / H) / H).astype(np.float32)
    cj = np.ones(WF, np.float32)
    cj[1:64] = 2.0
    Gc = (cj[:, None] * np.cos(2 * np.pi * np.outer(k2, w) / W) / W).astype(np.float32)
    Gs = (-cj[:, None] * np.sin(2 * np.pi * np.outer(k2, w) / W) / W).astype(np.float32)

    bf = ml_dtypes.bfloat16
    consts = {
        "R1": np.concatenate([Ch, Sh], 1),
        "R2a": np.concatenate([Cw, -Sw], 1),
        "R2b": np.concatenate([-Sw, -Cw], 1),
        "RA1": np.concatenate([Cih, Sih], 1),
        "RA2": np.concatenate([-Sih, Cih], 1),
        "RB1": Gc,
        "RB2": Gs,
    }
    G16 = np.zeros((128, 16), np.float32)
    E16 = np.zeros((16, 128), np.float32)
    for p in range(128):
        g = (p % 64) // 4
        G16[p, g] = 1.0
        E16[g, p] = 1.0
    F2 = np.zeros((128, 64), np.float32)
    for p in range(128):
        F2[p, p % 64] = 1.0 / HW
    E4 = np.zeros((4, 128), np.float32)
    for p in range(128):
        E4[p // 32, p] = 1.0
    maskJ = np.zeros((4, 128, 128), np.float32)
    for J in range(4):
        for p in range(128):
            f, pp = p // 32, p % 32
            maskJ[J, p, 32 * J + pp] = 1.0
    consts.update({"G16": G16, "E16": E16, "F2": F2, "E4": E4,
                   "maskJ": maskJ})
    return consts


def _prep_params(inputs):
    gamma = np.asarray(inputs["gn_gamma"], np.float64)
    beta = np.asarray(inputs["gn_beta"], np.float64)
    agg_w = np.asarray(inputs["agg_w"], np.float64)
    agg_b = np.asarray(inputs["agg_b"], np.float64)
    wg_w = np.asarray(inputs["wg_w"], np.float64)
    wg_b = np.asarray(inputs["wg_b"], np.float64)

    Wp = agg_w[:, :C] * gamma[None, :C] + agg_w[:, C:] * gamma[None, C:]
    bp = agg_w[:, :C] @ beta[:C] + agg_w[:, C:] @ beta[C:] + agg_b
    Wblk = np.zeros((128, 128), np.float32)
    for t in range(2):
        Wblk[64 * t:64 * t + 64, 64 * t:64 * t + 64] = Wp.T.astype(np.float32)
    bprime = np.zeros((128, 1), np.float32)
    bprime[:64, 0] = bp.astype(np.float32)
    bprime[64:, 0] = bp.astype(np.float32)
    WgT = wg_w.T.astype(np.float32)
    wgb = wg_b.astype(np.float32).reshape(1, 4)

    ds = np.asarray(inputs["ds_w"], np.float64)
    Wc = ds[..., 0] + 1j * ds[..., 1]                     # [F,C,H(k1),WF(k2)]
    rev = (-np.arange(H)) % H
    Wt = Wc.copy()
    for j in (0, WF - 1):
        Wt[..., j] = 0.5 * (Wc[..., j] + np.conj(Wc[:, :, rev, j]))
    rw = float(np.asarray(inputs["residual_weight"]).ravel()[0])
    return Wblk, bprime, WgT, wgb, Wt, rw


def _build_kernel():
    bf16, f32, f32r = dt.bfloat16, dt.float32, dt.float32r

    nc = bacc.Bacc("TRN2", target_bir_lowering=False, debug=False,
                   num_devices=NCORES)

    d = {}
    d["featf"] = nc.dram_tensor("featf", [128, B * CS * W], f32r,
                                kind="ExternalInput").ap()
    d["featg"] = nc.dram_tensor("featg", [128, 64 * 128], f32,
                                kind="ExternalInput").ap()
    d["ftiles"] = nc.dram_tensor("ftiles", [4, 128, CS * 2 * WF], f32r,
                                 kind="ExternalInput").ap()
    d["maskJ"] = nc.dram_tensor("maskJ", [4, 128, 128], f32,
                                kind="ExternalInput").ap()
    for name, shape, dty in [
        ("R1", [128, 256], f32r), ("R2a", [128, 130], f32),
        ("R2b", [128, 130], f32), ("RA1", [128, 256], f32r),
        ("RA2", [128, 256], f32r), ("RB1", [65, 128], f32),
        ("RB2", [65, 128], f32), ("G16", [128, 16], f32),
        ("E16", [16, 128], f32), ("F2", [128, 64], f32),
        ("E4", [4, 128], f32),
        ("Wblk", [128, 128], f32), ("bprime", [128, 1], f32),
        ("WgT", [64, 4], f32), ("wgb", [1, 4], f32),
        ("rcol", [128, 1], f32),
    ]:
        d[name] = nc.dram_tensor(name, shape, dty, kind="ExternalInput").ap()
    out_d = nc.dram_tensor("out", [B, CS, H, W], f32, kind="ExternalOutput").ap()
    if DEBUG:
        dbg = {
            "d_stats": nc.dram_tensor("d_stats", [128, 2], f32, kind="ExternalOutput").ap(),
            "d_gs": nc.dram_tensor("d_gs", [16, 6], f32, kind="ExternalOutput").ap(),
            "d_nstat": nc.dram_tensor("d_nstat", [128, 2], f32, kind="ExternalOutput").ap(),
            "d_gap": nc.dram_tensor("d_gap", [128, 16], f32, kind="ExternalOutput").ap(),
            "d_pooled": nc.dram_tensor("d_pooled", [64, 1], f32, kind="ExternalOutput").ap(),
            "d_logit": nc.dram_tensor("d_logit", [1, 8], f32, kind="ExternalOutput").ap(),
            "d_wrow": nc.dram_tensor("d_wrow", [1, 4], f32, kind="ExternalOutput").ap(),
            "d_wcol": nc.dram_tensor("d_wcol", [128, 8], f32, kind="ExternalOutput").ap(),
            "d_xn": nc.dram_tensor("d_xn", [128, 512], f32, kind="ExternalOutput").ap(),
            "d_y": nc.dram_tensor("d_y", [128, 512], f32, kind="ExternalOutput").ap(),
            "d_wb": nc.dram_tensor("d_wb", [128, 128], f32, kind="ExternalOutput").ap(),
            "d_wb2": nc.dram_tensor("d_wb2", [128, 128], f32, kind="ExternalOutput").ap(),
        }

    with tile.TileContext(nc) as tc:
        with (
            tc.tile_pool(name="consts", bufs=1) as cp,
            tc.tile_pool(name="feat", bufs=1) as fp,
            tc.tile_pool(name="gate", bufs=1) as gp,
            tc.tile_pool(name="work", bufs=3) as wkp,
            tc.tile_pool(name="sgrp", bufs=2) as sgp,
            tc.tile_pool(name="outp", bufs=3) as op_,
            tc.tile_pool(name="ps_a", bufs=2, space="PSUM") as ps_a,
            tc.tile_pool(name="ps_b", bufs=2, space="PSUM") as ps_b,
            tc.tile_pool(name="ps_c", bufs=2, space="PSUM") as ps_c,
            tc.tile_pool(name="ps_d", bufs=1, space="PSUM") as ps_d,
            tc.tile_pool(name="ps_m", bufs=1, space="PSUM") as ps_m,
            tc.tile_pool(name="dram", bufs=1, space="DRAM") as dr,
        ):
            ct = {}
            for name in ["R1", "R2a", "R2b", "RA1", "RA2", "RB1", "RB2",
                         "G16", "E16", "F2", "E4", "Wblk",
                         "bprime", "WgT", "wgb", "rcol"]:
                t = cp.tile(list(d[name].shape), d[name].dtype, tag=name)
                nc.sync.dma_start(t[:], d[name][:])
                ct[name] = t
            for name in ["R2a", "R2b", "RB1", "RB2"]:
                t = cp.tile(list(d[name].shape), bf16, tag=name + "b")
                nc.vector.tensor_copy(t[:], ct[name][:])
                ct[name] = t

            maskt = []
            for J in range(4):
                t = cp.tile([128, 128], f32, tag=f"maskJ{J}")
                nc.sync.dma_start(t[:], d["maskJ"][J])
                maskt.append(t)

            featb = []
            for b in range(B):
                t = fp.tile([128, CS * W], f32r, tag=f"featb{b}")
                nc.sync.dma_start(t[:], d["featf"][:, b * CS * W:(b + 1) * CS * W])
                featb.append(t)
            featg = fp.tile([128, 64 * 128], f32, tag="featg")
            nc.sync.dma_start(featg[:], d["featg"][:])
            ftl = []
            for J in range(4):
                t = fp.tile([128, CS * 2 * WF], f32r, tag=f"ftl{J}")
                nc.sync.dma_start(t[:], d["ftiles"][J])
                ftl.append(t)

            # ================= gating (sample b = core id) ===================
            xn = gp.tile([128, 64 * 128], f32r, tag="xn")
            wblkr = gp.tile([128, 128], f32r, tag="wblkr")
            nc.vector.tensor_copy(wblkr[:], ct["Wblk"][:])
            stats = gp.tile([128, 2], f32, tag="stats")
            nc.vector.tensor_scalar(xn[:], featg[:], 1.0, 0.0, ALU.mult,
                                    ALU.add, accum_out=stats[:, 0:1])
            nc.scalar.activation(xn[:], featg[:], AF.Square,
                                 accum_out=stats[:, 1:2])
            gstat = ps_m.tile([16, 2], f32, tag="pmix")
            nc.tensor.matmul(gstat[:], ct["G16"][:], stats[:])
            gs = gp.tile([16, 6], f32, tag="gs")
            nc.scalar.mul(gs[:, 0:1], gstat[:, 0:1], 1.0 / (4 * HW))
            nc.scalar.mul(gs[:, 1:2], gstat[:, 1:2], 1.0 / (4 * HW))
            nc.scalar.activation(gs[:, 2:3], gs[:, 0:1], AF.Square)
            nc.vector.tensor_sub(gs[:, 3:4], gs[:, 1:2], gs[:, 2:3])
            epst = gp.tile([16, 1], f32, tag="epst")
            nc.vector.memset(epst[:], EPS)
            nc.scalar.activation(gs[:, 4:5], gs[:, 3:4], AF.Sqrt,
                                 bias=epst[:, 0:1])
            nc.vector.reciprocal(gs[:, 5:6], gs[:, 4:5])
            gs2 = gp.tile([16, 2], f32, tag="gs2")
            nc.vector.tensor_mul(gs2[:, 0:1], gs[:, 0:1], gs[:, 5:6])
            nc.vector.tensor_scalar_mul(gs2[:, 0:1], gs2[:, 0:1], -1.0)
            nc.vector.tensor_copy(gs2[:, 1:2], gs[:, 5:6])
            pstat = ps_m.tile([128, 2], f32, tag="pmix")
            nc.tensor.matmul(pstat[:], ct["E16"][:], gs2[:])
            nstat = gp.tile([128, 2], f32, tag="nstat")
            nc.scalar.copy(nstat[:], pstat[:])
            nc.scalar.activation(xn[:], featg[:], AF.Identity,
                                 bias=nstat[:, 0:1], scale=nstat[:, 1:2])
            if DEBUG:
                wbf = gp.tile([128, 128], f32, tag="wbf")
                nc.vector.tensor_copy(wbf[:], wblkr[:])
                nc.sync.dma_start(dbg["d_wb"][:], wbf[:])
                xnf = gp.tile([128, 512], f32, tag="xnf")
                nc.vector.tensor_copy(xnf[:], xn[:, 0:512])
                nc.sync.dma_start(dbg["d_xn"][:], xnf[:])
            gap = gp.tile([128, 16], f32, tag="gap")
            for j in range(16):
                yp = ps_a.tile([128, 512], f32, tag="p1")
                nc.tensor.matmul(yp[:], wblkr[:],
                                 xn[:, j * 512:(j + 1) * 512])
                nc.scalar.activation(
                    featg[:, j * 512:(j + 1) * 512], yp[:], AF.Relu,
                    bias=ct["bprime"][:, 0:1], scale=1.0,
                    accum_out=gap[:, j:j + 1])
                if DEBUG and j == 0:
                    yf = gp.tile([128, 512], f32, tag="yf")
                    nc.scalar.copy(yf[:], yp[:])
                    nc.sync.dma_start(dbg["d_y"][:], yf[:])
            gsum = gp.tile([128, 1], f32, tag="gsum")
            nc.vector.tensor_reduce(gsum[:], gap[:], AX.X, ALU.add)
            ppool = ps_m.tile([64, 1], f32, tag="pmix")
            nc.tensor.matmul(ppool[:], ct["F2"][:], gsum[:])
            pooled = gp.tile([64, 1], f32, tag="pooled")
            nc.scalar.copy(pooled[:], ppool[:])
            plog = ps_m.tile([1, 4], f32, tag="pmix")
            nc.tensor.matmul(plog[:], pooled[:], ct["WgT"][:])
            logit = gp.tile([1, 8], f32, tag="logit")
            nc.vector.memset(logit[:], 0.0)
            nc.vector.tensor_add(logit[:, 0:4], plog[:], ct["wgb"][:])
            nc.vector.tensor_reduce(logit[:, 4:5], logit[:, 0:4], AX.X, ALU.max)
            nc.vector.tensor_scalar(logit[:, 0:4], logit[:, 0:4],
                                    logit[:, 4:5], None, ALU.subtract)
            wrow = gp.tile([1, 4], f32, tag="wrow")
            nc.scalar.activation(wrow[:], logit[:, 0:4], AF.Exp,
                                 accum_out=logit[:, 5:6])
            nc.vector.reciprocal(logit[:, 6:7], logit[:, 5:6])
            nc.vector.tensor_scalar(wrow[:], wrow[:], logit[:, 6:7], None,
                                    ALU.mult)
            ag_in = dr.tile([1, 4], f32)
            ag_out = dr.tile([8, 4], f32)
            nc.sync.dma_start(ag_in[:], wrow[:])
            if SIM_MODE:
                for _b in range(8):
                    nc.sync.dma_start(ag_out[_b:_b + 1, :], ag_in[:])
            else:
                nc.gpsimd.collective_compute(
                    "AllGather", ALU.bypass, ins=[ag_in.opt()],
                    outs=[ag_out.opt()],
                    replica_groups=[list(range(NCORES))],
                )
            wT = gp.tile([4, 8], f32, tag="wT")
            nc.sync.dma_start(wT[:], ag_out[:].rearrange("b f -> f b"))
            pwcol = ps_m.tile([128, 8], f32, tag="pmix")
            nc.tensor.matmul(pwcol[:], ct["E4"][:], wT[:])
            wcol = gp.tile([128, 8], f32, tag="wcol")
            nc.scalar.copy(wcol[:], pwcol[:])
            if DEBUG:
                nc.sync.dma_start(dbg["d_stats"][:], stats[:])
                nc.sync.dma_start(dbg["d_gs"][:], gs[:])
                nc.sync.dma_start(dbg["d_nstat"][:], nstat[:])
                nc.sync.dma_start(dbg["d_gap"][:], gap[:])
                nc.sync.dma_start(dbg["d_pooled"][:], pooled[:])
                nc.sync.dma_start(dbg["d_logit"][:], logit[:])
                nc.sync.dma_start(dbg["d_wrow"][:], wrow[:])
                nc.sync.dma_start(dbg["d_wcol"][:], wcol[:])
            wpat = []
            for b in range(B):
                row = []  # d_wb2 dump appended after wpat build below
                for J in range(4):
                    t = gp.tile([128, 128], f32r, tag=f"wpat{b}_{J}")
                    nc.scalar.activation(t[:], maskt[J][:], AF.Identity,
                                         scale=wcol[:, b:b + 1])
                    row.append(t)
                wpat.append(row)

            if DEBUG:
                zz = gp.tile([128, 128], bf16, tag="zz")
                nc.vector.tensor_scalar(zz[:], wpat[7][3][:], 0.0, None, ALU.mult)
                wb2 = gp.tile([128, 128], f32, tag="wb2")
                nc.vector.tensor_add(wb2[:], ct["Wblk"][:], zz[:])
                nc.sync.dma_start(dbg["d_wb2"][:], wb2[:])
            # ================= FFT branch ====================================
            for b in range(N_B):
                fb = featb[b]
                for g in range(2):                      # 4-chain groups
                    c0 = 4 * g
                    Sr4 = sgp.tile([128, 260], f32, tag="Sr4")
                    Si4 = sgp.tile([128, 260], f32, tag="Si4")
                    Xi4 = sgp.tile([128, 260], f32, tag="Xi4")
                    Dr4 = sgp.tile([128, 260], f32r, tag="Dr4")
                    Di4 = sgp.tile([128, 260], f32r, tag="Di4")
                    Wm4 = sgp.tile([128, 520], f32, tag="Wm4")
                    m1 = sgp.tile([128, 260], f32, tag="m1")
                    m2 = sgp.tile([128, 260], f32, tag="m2")
                    m3 = sgp.tile([128, 260], f32, tag="m3")
                    m4 = sgp.tile([128, 260], f32, tag="m4")
                    pB = ps_d.tile([128, 512], f32, tag="pB")
                    for cc in range(2):                 # 2-chain psum subgroups
                        ch2 = c0 + 2 * cc
                        pm = ps_m.tile([128, 260], f32, tag="pmix")
                        for J in range(4):
                            nc.tensor.matmul(
                                pm[:], wpat[b][J][:],
                                ftl[J][:, ch2 * 130:(ch2 + 2) * 130],
                                start=(J == 0), stop=(J == 3))
                        p1 = ps_a.tile([128, 512], f32, tag="p1")
                        for j in range(2):
                            c = ch2 + j
                            nc.tensor.matmul(p1[:, j * 256:(j + 1) * 256],
                                             fb[:, c * 128:(c + 1) * 128],
                                             ct["R1"][:])
                        uv = wkp.tile([128, 512], bf16, tag="uv")
                        if cc == 0:
                            nc.vector.tensor_copy(uv[:], p1[:])
                        else:
                            nc.scalar.copy(uv[:], p1[:])
                        p2 = ps_b.tile([128, 260], f32, tag="p2")
                        for j in range(2):
                            nc.tensor.matmul(p2[:, j * 130:(j + 1) * 130],
                                             uv[:, j * 256:j * 256 + 128],
                                             ct["R2a"][:], start=True,
                                             stop=False)
                            nc.tensor.matmul(p2[:, j * 130:(j + 1) * 130],
                                             uv[:, j * 256 + 128:(j + 1) * 256],
                                             ct["R2b"][:], start=False,
                                             stop=True)
                        # strided views: [128, 2chain, 65]
                        p2v = p2[:].rearrange("p (j x) -> p j x", j=2)
                        xr = p2v[:, :, 0:65]
                        xi = p2v[:, :, 65:130]
                        s4 = slice(2 * cc, 2 * cc + 2)
                        srv = Sr4[:].rearrange("p (q x) -> p q x", q=4)[:, s4]
                        siv = Si4[:].rearrange("p (q x) -> p q x", q=4)[:, s4]
                        xiv = Xi4[:].rearrange("p (q x) -> p q x", q=4)[:, s4]
                        m1v = m1[:].rearrange("p (q x) -> p q x", q=4)[:, s4]
                        nc.scalar.activation(srv, xr, AF.Square)   # Xr^2
                        nc.vector.tensor_copy(xiv, xi)             # Xi
                        nc.scalar.activation(m1v, xi, AF.Square)   # Xi^2 (scratch)
                        # Si = 2*Xr*Xi  (one psum operand)
                        nc.vector.scalar_tensor_tensor(siv, xr, 2.0, xiv,
                                                       ALU.mult, ALU.mult)
                        # Sr = Xr^2 - Xi^2
                        nc.vector.tensor_sub(srv, srv, m1v)
                        nc.scalar.copy(Wm4[:, cc * 260:(cc + 1) * 260], pm[:])
                    # ---- D = S * Wmix  (4 chains batched) ----
                    wmv = Wm4[:].rearrange("p (q x) -> p q x", q=4)
                    wmr = wmv[:, :, 0:65]
                    wmi = wmv[:, :, 65:130]
                    sr_f = Sr4[:].rearrange("p (q x) -> p q x", q=4)
                    si_f = Si4[:].rearrange("p (q x) -> p q x", q=4)
                    m1f = m1[:].rearrange("p (q x) -> p q x", q=4)
                    m2f = m2[:].rearrange("p (q x) -> p q x", q=4)
                    m3f = m3[:].rearrange("p (q x) -> p q x", q=4)
                    m4f = m4[:].rearrange("p (q x) -> p q x", q=4)
                    nc.vector.tensor_mul(m1f, sr_f, wmr)
                    nc.vector.tensor_mul(m2f, si_f, wmi)
                    nc.gpsimd.tensor_mul(m3f, sr_f, wmi)
                    nc.gpsimd.tensor_mul(m4f, si_f, wmr)
                    nc.vector.tensor_sub(Dr4[:], m1[:], m2[:])
                    nc.gpsimd.tensor_add(Di4[:], m3[:], m4[:])
                    # ---- iDFT ----
                    for cc in range(2):
                        pA = ps_c.tile([65, 512], f32, tag="pA")
                        for j in range(2):
                            q = 2 * cc + j
                            nc.tensor.matmul(pA[:, j * 256:(j + 1) * 256],
                                             Dr4[:, q * 65:(q + 1) * 65],
                                             ct["RA1"][:], start=True,
                                             stop=False)
                            nc.tensor.matmul(pA[:, j * 256:(j + 1) * 256],
                                             Di4[:, q * 65:(q + 1) * 65],
                                             ct["RA2"][:], start=False,
                                             stop=True)
                        z2 = wkp.tile([65, 512], bf16, tag="z2")
                        if cc == 0:
                            nc.vector.tensor_copy(z2[:], pA[:])
                        else:
                            nc.scalar.copy(z2[:], pA[:])
                        for j in range(2):
                            q = 2 * cc + j
                            nc.tensor.matmul(pB[:, q * 128:(q + 1) * 128],
                                             z2[:, j * 256:j * 256 + 128],
                                             ct["RB1"][:], start=True,
                                             stop=False)
                            nc.tensor.matmul(pB[:, q * 128:(q + 1) * 128],
                                             z2[:, j * 256 + 128:(j + 1) * 256],
                                             ct["RB2"][:], start=False,
                                             stop=True)
                    ot = op_.tile([128, 512], f32, tag="ot")
                    nc.vector.scalar_tensor_tensor(
                        ot[:], fb[:, c0 * 128:(c0 + 4) * 128].bitcast(f32),
                        ct["rcol"][:, 0:1], pB[:], ALU.mult, ALU.add)
                    nc.sync.dma_start(
                        out_d[b, c0:c0 + 4].rearrange("c h w -> h c w"),
                        ot[:].rearrange("p (c w) -> p c w", c=4))
    nc.compile()
    return nc


def _get_kernel():
    if "nc" not in _cache:
        _cache["nc"] = _build_kernel()
        _cache["consts"] = _build_constants()
    return _cache["nc"], _cache["consts"]


def kernel(**inputs):
    nc, consts = _get_kernel()
    Wblk, bprime, WgT, wgb, Wt, rw = _prep_params(inputs)
    feat = np.asarray(inputs["features"], np.float32)
    bf = ml_dtypes.bfloat16

    rcol = np.full((128, 1), rw, np.float32)
    base = {
        "R1": consts["R1"], "R2a": consts["R2a"], "R2b": consts["R2b"],
        "RA1": consts["RA1"], "RA2": consts["RA2"], "RB1": consts["RB1"],
        "RB2": consts["RB2"], "G16": consts["G16"], "E16": consts["E16"],
        "F2": consts["F2"], "E4": consts["E4"], "maskJ": consts["maskJ"],
        "Wblk": Wblk, "bprime": bprime, "WgT": WgT, "wgb": wgb,
        "rcol": rcol,
    }
    in_maps = []
    for k in range(NCORES):
        sl = slice(k * CS, (k + 1) * CS)
        # featf: [h, (b, c, w)]
        ff = feat[:, sl].transpose(2, 0, 1, 3).reshape(128, B * CS * W).copy()
        # featg: [(t, c), (s)] with t = h-half
        fg = feat[k].reshape(C, 2, 64 * 128).transpose(1, 0, 2) \
                    .reshape(128, 64 * 128).copy()
        # ftiles: [J, (f, p), (c, ri, k2)]
        Wts = Wt[:, sl]                                   # [F, CS, H, WF]
        ftiles = np.empty((4, 128, CS * 2 * WF), np.float32)
        for J in range(4):
            blk = Wts[:, :, 32 * J:32 * J + 32, :]        # [F, CS, 32, WF]
            re = blk.real.astype(np.float32)
            im = blk.imag.astype(np.float32)
            # [(f,p), (c, ri, k2)]
            stacked = np.stack([re, im], axis=3)          # [F, CS, 32, 2, WF]
            ftiles[J] = stacked.transpose(0, 2, 1, 3, 4).reshape(128, CS * 2 * WF)
        m = dict(base)
        m["featf"] = ff
        m["featg"] = fg
        m["ftiles"] = ftiles
        in_maps.append(m)

    res = run_bass_kernel_spmd(nc, in_maps, list(range(NCORES)))
    out = np.empty((B, C, H, W), np.float32)
    for k in range(NCORES):
        out[:, k * CS:(k + 1) * CS] = res.results[k]["out"]
    return out


if __name__ == "__main__":
    import jax
    jax.config.update("jax_platforms", "cpu")



# revision 27
# speedup vs baseline: 1.5209x; 1.5209x over previous
"""Trainium2 Bass kernel for nn_FDSM_40295383171690 (v2, restructured).

Math (validated in numpy, rel err ~0.6%):
  gating: GN(concat(x,x)) == concat(GN4(x), GN4(x)); gamma/beta folded into
          1x1 conv (Wblk, bp); GN normalization folded into the conv weights
          (scale rows by rstd, bias -= W.(mu*rstd)) so xn is never
          materialized. weights = softmax(wg @ GAP(relu(...)))
  fft:    out = irfft2( rfft2(x)^2 * Wmix ) + r*x, Wmix = sum_f w[b,f] Wsym[f]
          stage2 emits tmpA = Xr+Xi, tmpB = Xr-Xi directly (modified DFT
          matrices), so Sr = tmpA*tmpB and Si = (tmpA/sqrt2)^2 - (tmpB/sqrt2)^2.

Phasing: gating + AllGather overlap with the weight-independent forward DFT
(stage1/stage2/S) for all 64 chains; the weight-dependent half (Wmix matmul,
complex product, inverse DFT) runs after, with the core's own sample's
weights available ~15us before the collective completes.

Sharding: core k = gating for sample k (all C) + FFT for channels
[8k,8k+8) of all samples; the [8,4] gating weights are AllGathered on-chip.
"""

import numpy as np
import ml_dtypes

import concourse.bass as bass
import concourse.bacc as bacc
import concourse.mybir as mybir
import concourse.tile as tile
from concourse.bass_utils import run_bass_kernel_spmd

dt = mybir.dt
AF = mybir.ActivationFunctionType
ALU = mybir.AluOpType
AX = mybir.AxisListType

B, C, H, W, F = 8, 64, 128, 128, 4
WF = 65
NCORES = 8
CS = C // NCORES
EPS = 1e-5
HW = H * W

NA = 1380          # packA (f32) columns
NB = 1028          # packB (bf16) columns
# packA column offsets
A_R1, A_RA1, A_RA2, A_WBLK = 0, 256, 512, 768
A_F2, A_G16, A_BP = 896, 960, 976
A_E16, A_E4, A_WGT, A_WGB, A_RCOL = 978, 1106, 1234, 1238, 1242
A_PK = 1244        # [8, 8] per-core sample-rotation permutation (rows 0:8)
A_RI = 1252        # residual_weight * I[128] for the residual-add matmul
# packB column offsets
B_R2A, B_R2B, B_MASK, B_RB1, B_RB2 = 0, 130, 260, 772, 900

_cache = {}


def _build_constants():
    h = np.arange(H)
    k1 = np.arange(H)
    w = np.arange(W)
    k2 = np.arange(WF)
    Ch = np.cos(2 * np.pi * np.outer(h, k1) / H).astype(np.float32)
    Sh = np.sin(2 * np.pi * np.outer(h, k1) / H).astype(np.float32)
    Cw = np.cos(2 * np.pi * np.outer(w, k2) / W).astype(np.float32)
    Sw = np.sin(2 * np.pi * np.outer(w, k2) / W).astype(np.float32)
    Cih = (np.cos(2 * np.pi * np.outer(k1, h) / H) / H).astype(np.float32)
    Sih = (np.sin(2 * np.pi * np.outer(k1, h) / H) / H).astype(np.float32)
    cj = np.ones(WF, np.float32)
    cj[1:64] = 2.0
    Gc = (cj[:, None] * np.cos(2 * np.pi * np.outer(k2, w) / W) / W).astype(np.float32)
    Gs = (-cj[:, None] * np.sin(2 * np.pi * np.outer(k2, w) / W) / W).astype(np.float32)

    G16 = np.zeros((128, 16), np.float32)
    E16 = np.zeros((16, 128), np.float32)
    for p in range(128):
        g = (p % 64) // 4
        G16[p, g] = 1.0
        E16[g, p] = 1.0
    F2 = np.zeros((128, 64), np.float32)
    for p in range(128):
        F2[p, p % 64] = 1.0 / HW
    # mask block J: [p, k1] nonzero at k1 = 32J + p%32 (for every f-block row)
    maskcat = np.zeros((128, 512), np.float32)
    for J in range(4):
        for p in range(128):
            pp = p % 32
            maskcat[p, 128 * J + 32 * J + pp] = 1.0
    E4 = np.zeros((4, 128), np.float32)
    for p in range(128):
        E4[p // 32, p] = 1.0

    return {
        "E4": E4,
        "R1": np.concatenate([Ch, Sh], 1),
        "RA1": np.concatenate([Cih, Sih], 1),
        "RA2": np.concatenate([-Sih, Cih], 1),
        "R2ap": np.concatenate([Cw - Sw, Cw + Sw], 1),       # [128,130]
        "R2bp": np.concatenate([-Sw - Cw, Cw - Sw], 1),
        "RB1": Gc, "RB2": Gs,
        "G16": G16, "E16": E16, "F2": F2, "maskcat": maskcat,
    }


def _prep_params(inputs):
    gamma = np.asarray(inputs["gn_gamma"], np.float64)
    beta = np.asarray(inputs["gn_beta"], np.float64)
    agg_w = np.asarray(inputs["agg_w"], np.float64)
    agg_b = np.asarray(inputs["agg_b"], np.float64)
    wg_w = np.asarray(inputs["wg_w"], np.float64)
    wg_b = np.asarray(inputs["wg_b"], np.float64)

    Wp = agg_w[:, :C] * gamma[None, :C] + agg_w[:, C:] * gamma[None, C:]
    bp = agg_w[:, :C] @ beta[:C] + agg_w[:, C:] @ beta[C:] + agg_b
    Wblk = np.zeros((128, 128), np.float32)
    for t in range(2):
        Wblk[64 * t:64 * t + 64, 64 * t:64 * t + 64] = Wp.T.astype(np.float32)
    bp_col = np.zeros((128,), np.float32)
    bp_col[:64] = bp
    bp_col[64:] = bp
    WgT = wg_w.T.astype(np.float32)                      # [64, 4]
    wgb = wg_b.astype(np.float32)

    ds = np.asarray(inputs["ds_w"], np.float64)
    Wc = ds[..., 0] + 1j * ds[..., 1]                    # [F,C,H,WF]
    rev = (-np.arange(H)) % H
    Wt = Wc.copy()
    for j in (0, WF - 1):
        Wt[..., j] = 0.5 * (Wc[..., j] + np.conj(Wc[:, :, rev, j]))
    rw = float(np.asarray(inputs["residual_weight"]).ravel()[0])
    return Wblk, bp_col, WgT, wgb, Wt, rw


def _build_kernel():
    bf16, f32, f32r = dt.bfloat16, dt.float32, dt.float32r

    nc = bacc.Bacc("TRN2", target_bir_lowering=False, debug=False,
                   num_devices=NCORES)

    packA_d = nc.dram_tensor("packA", [128, NA], f32r, kind="ExternalInput").ap()
    packB_d = nc.dram_tensor("packB", [128, NB], bf16, kind="ExternalInput").ap()
    featg_d = nc.dram_tensor("featg", [128, 8192], bf16, kind="ExternalInput").ap()
    featf_d = nc.dram_tensor("featf", [128, 8192], f32r, kind="ExternalInput").ap()
    ftl_d = nc.dram_tensor("ftl", [4, 128, CS * 2 * WF], bf16, kind="ExternalInput").ap()
    out_d = nc.dram_tensor("out", [B, 128, CS * W], f32, kind="ExternalOutput").ap()

    with tile.TileContext(nc) as tc:
        with (
            tc.tile_pool(name="cp", bufs=1) as cp,
            tc.tile_pool(name="fp", bufs=1) as fp,
            tc.tile_pool(name="gp", bufs=1) as gp,
            tc.tile_pool(name="wpp", bufs=1) as wpp,
            tc.tile_pool(name="uvp", bufs=3) as uvp,
            tc.tile_pool(name="qp", bufs=3) as qp,
            tc.tile_pool(name="srp", bufs=32) as srp,
            tc.tile_pool(name="mp", bufs=3) as mp,
            tc.tile_pool(name="drp", bufs=3) as drp,
            tc.tile_pool(name="z2p", bufs=3) as z2p,
            tc.tile_pool(name="otp", bufs=3) as otp,
            tc.tile_pool(name="ps_a", bufs=3, space="PSUM") as ps_a,
            tc.tile_pool(name="ps_m", bufs=2, space="PSUM") as ps_m,
            tc.tile_pool(name="ps_o", bufs=3, space="PSUM") as ps_o,
            tc.tile_pool(name="dram", bufs=1, space="DRAM") as dr,
        ):
            # ---- input DMAs (SP queue order == emission order) ----
            pkA = cp.tile([128, NA], f32r, tag="pkA")
            nc.sync.dma_start(pkA[:], packA_d[:])
            gq = []
            for j in range(4):
                t = cp.tile([128, 2048], bf16, tag=f"gq{j}")
                nc.sync.dma_start(t[:], featg_d[:, j * 2048:(j + 1) * 2048])
                gq.append(t)
            pkB = cp.tile([128, NB], bf16, tag="pkB")
            nc.sync.dma_start(pkB[:], packB_d[:])

            # core id (sample owned by this core) is a runtime input; but the
            # program itself is compiled per-core (SPMD with per-core inputs),
            # so we bake the rotation into the host-side input layout instead:
            # featf column block i holds sample bs[i] = (k + i) % 8.
            fb = []
            ftl = [None] * 4
            for i in range(B):
                t = fp.tile([128, CS * W], f32r, tag=f"fb{i}")
                nc.sync.dma_start(t[:], featf_d[:, i * 1024:(i + 1) * 1024])
                fb.append(t)
                if i == 1:
                    for J in range(4):
                        tt = fp.tile([128, CS * 2 * WF], bf16, tag=f"ftl{J}")
                        nc.sync.dma_start(tt[:], ftl_d[J])
                        ftl[J] = tt

            # const views
            R1v = pkA[:, A_R1:A_R1 + 256]
            RA1v = pkA[:, A_RA1:A_RA1 + 256]
            RA2v = pkA[:, A_RA2:A_RA2 + 256]
            Wblkv = pkA[:, A_WBLK:A_WBLK + 128].bitcast(f32)
            F2v = pkA[:, A_F2:A_F2 + 64]
            G16v = pkA[:, A_G16:A_G16 + 16]
            bpv = pkA[:, A_BP:A_BP + 1].bitcast(f32)
            E16v = pkA[0:16, A_E16:A_E16 + 128]
            E4v = pkA[0:4, A_E4:A_E4 + 128]
            WgTv = pkA[0:64, A_WGT:A_WGT + 4]
            wgbv = pkA[0:1, A_WGB:A_WGB + 4].bitcast(f32)
            rcolv = pkA[:, A_RCOL:A_RCOL + 1].bitcast(f32)
            Pkv = pkA[0:8, A_PK:A_PK + 8]
            RIv = pkA[:, A_RI:A_RI + 128]
            R2aL = pkB[:, B_R2A:B_R2A + 65]
            R2aR = pkB[:, B_R2A + 65:B_R2A + 130]
            R2bL = pkB[:, B_R2B:B_R2B + 65]
            R2bR = pkB[:, B_R2B + 65:B_R2B + 130]
            maskv = pkB[:, B_MASK:B_MASK + 512]
            RB1v = pkB[0:65, B_RB1:B_RB1 + 128]
            RB2v = pkB[0:65, B_RB2:B_RB2 + 128]

            # ---- act table preload (sqrt set covers square/relu/identity) ----
            epst = gp.tile([16, 1], f32, tag="epst")
            nc.vector.memset(epst[:], EPS)
            dumm = gp.tile([16, 1], f32, tag="dumm")
            nc.scalar.activation(dumm[:], epst[:], AF.Sqrt)

            # =================== gating (sample k) ===================
            stats = gp.tile([128, 8], f32, tag="stats")
            scrA = gp.tile([128, 2048], bf16, tag="scrA")
            scrD = gp.tile([128, 2048], bf16, tag="scrD")
            # sums on DVE (tensor_scalar 4x), squares: act q0,q1,q2; DVE q3
            for j in range(4):
                nc.vector.tensor_scalar(scrD[:], gq[j][:], 1.0, 0.0, ALU.mult,
                                        ALU.add,
                                        accum_out=stats[:, 4 + j:5 + j])
            for j in range(3):
                nc.scalar.activation(scrA[:], gq[j][:], AF.Square,
                                     accum_out=stats[:, j:j + 1])
            nc.vector.tensor_tensor_reduce(
                scrD[:], gq[3][:], gq[3][:], 1.0, 0.0, ALU.mult, ALU.add,
                accum_out=stats[:, 3:4])
            stats2 = gp.tile([128, 2], f32r, tag="stats2")
            with nc.allow_low_precision(reason="f32r staging for matmul rhs"):
                nc.vector.tensor_reduce(stats2[:, 0:1], stats[:, 4:8], AX.X,
                                        ALU.add)
                nc.vector.tensor_reduce(stats2[:, 1:2], stats[:, 0:4], AX.X,
                                        ALU.add)

            psml = ps_m.tile([128, 512], f32, tag="mid")   # shared small-outs
            gstat = psml[0:16, 0:2]
            nc.tensor.matmul(gstat, G16v, stats2[:])
            gs = gp.tile([16, 8], f32, tag="gs")
            nc.vector.tensor_scalar_mul(gs[:, 0:1], gstat[:, 0:1], 1.0 / 65536.0)
            nc.vector.tensor_scalar_mul(gs[:, 1:2], gstat[:, 1:2], 1.0 / 65536.0)
            nc.vector.tensor_mul(gs[:, 2:3], gs[:, 0:1], gs[:, 0:1])
            nc.vector.tensor_sub(gs[:, 3:4], gs[:, 1:2], gs[:, 2:3])
            nc.scalar.activation(gs[:, 4:5], gs[:, 3:4], AF.Sqrt,
                                 bias=epst[:, 0:1])
            nc.vector.reciprocal(gs[:, 5:6], gs[:, 4:5])
            nc.vector.tensor_mul(gs[:, 6:7], gs[:, 0:1], gs[:, 5:6])
            gsr = gp.tile([16, 2], f32r, tag="gsr")
            nc.vector.tensor_copy(gsr[:], gs[:, 5:7])
            psml2 = ps_m.tile([128, 512], f32, tag="mid")
            nstat = psml2[:, 0:2]
            nc.tensor.matmul(nstat, E16v, gsr[:])
            nstat_sb = gp.tile([128, 2], f32, tag="nstat")
            nc.vector.tensor_copy(nstat_sb[:], nstat)
            mus_bf = gp.tile([128, 1], bf16, tag="mus")
            nc.vector.tensor_copy(mus_bf[:], nstat_sb[:, 1:2])
            Wbs = gp.tile([128, 128], bf16, tag="Wbs")
            nc.scalar.activation(Wbs[:], Wblkv, AF.Identity,
                                 scale=nstat_sb[:, 0:1])
            psml3 = ps_m.tile([128, 512], f32, tag="mid")
            pbias = psml3[:, 0:1]
            nc.tensor.matmul(pbias, Wbs[:], mus_bf[:])
            bvec = gp.tile([128, 1], f32, tag="bvec")
            nc.vector.tensor_sub(bvec[:], bpv, pbias)

            gap = gp.tile([128, 16], f32, tag="gap")
            scr5a = gp.tile([128, 512], bf16, tag="scr5a")
            scr5d = gp.tile([128, 512], bf16, tag="scr5d")
            scr5p = gp.tile([128, 512], bf16, tag="scr5p")
            for j in range(16):
                yp = ps_m.tile([128, 512], f32, tag="mid")
                nc.tensor.matmul(yp[:], Wbs[:],
                                 gq[j // 4][:, (j % 4) * 512:(j % 4 + 1) * 512])
                if j % 2 == 0:
                    nc.scalar.activation(scr5a[:], yp[:], AF.Relu,
                                         bias=bvec[:, 0:1],
                                         accum_out=gap[:, j:j + 1])
                else:
                    nc.vector.tensor_scalar(scr5d[:], yp[:], bvec[:, 0:1], 0.0,
                                            ALU.add, ALU.max,
                                            accum_out=gap[:, j:j + 1])
            gsum = gp.tile([128, 2], f32r, tag="gsum")
            nc.vector.tensor_scalar_mul(gsum[:, 1:2], gap[:, 0:1], 0.0)
            with nc.allow_low_precision(reason="f32r staging for matmul rhs"):
                nc.vector.tensor_reduce(gsum[:, 0:1], gap[:], AX.X, ALU.add)
            psml4 = ps_m.tile([128, 512], f32, tag="mid")
            pooled = psml4[0:64, 0:2]
            nc.tensor.matmul(pooled, F2v, gsum[:])
            pooled_sb = gp.tile([64, 1], f32r, tag="pooled")
            nc.vector.tensor_copy(pooled_sb[:], pooled[:, 0:1])
            psml5 = ps_m.tile([128, 512], f32, tag="mid")
            plog = psml5[0:1, 0:4]
            nc.tensor.matmul(plog, pooled_sb[:], WgTv)
            lg = gp.tile([1, 8], f32, tag="lg")
            nc.vector.tensor_add(lg[:, 0:4], plog, wgbv)
            nc.vector.tensor_reduce(lg[:, 4:5], lg[:, 0:4], AX.X, ALU.max)
            nc.vector.tensor_scalar_mul(lg[:, 5:6], lg[:, 4:5], -1.0)
            wrow = gp.tile([1, 4], f32r, tag="wrow")
            nc.scalar.activation(wrow[:], lg[:, 0:4], AF.Exp,
                                 bias=lg[:, 5:6], accum_out=lg[:, 6:7])
            nc.vector.reciprocal(lg[:, 7:8], lg[:, 6:7])
            nc.vector.tensor_scalar(wrow[:], wrow[:], lg[:, 7:8], None, ALU.mult)

            ag_in = dr.tile([1, 4], f32r)
            ag_out = dr.tile([8, 4], f32r)
            nc.sync.dma_start(ag_in[:], wrow[:])
            # own-sample weights available immediately
            wT_own = gp.tile([4, 2], f32r, tag="wTown")
            nc.vector.tensor_scalar_mul(wT_own[:, 1:2], epst[0:4, 0:1], 0.0)
            nc.sync.dma_start(wT_own[:, 0:1], ag_in[:].rearrange("a f -> f a"))
            psml6 = ps_m.tile([128, 512], f32, tag="mid")
            pwown = psml6[:, 0:2]
            nc.tensor.matmul(pwown, E4v, wT_own[:])
            wcol = gp.tile([128, 8], f32, tag="wcol")
            nc.vector.tensor_copy(wcol[:, 0:1], pwown[:, 0:1])
            wpat = [None] * B
            # wpat index i corresponds to fb[i] (host maps sample (k+i)%8)
            wpat[0] = wpp.tile([128, 512], bf16, tag="wpat0", name="wpat0")
            nc.vector.tensor_scalar(wpat[0][:], maskv, wcol[:, 0:1], None,
                                    ALU.mult)

            nc.gpsimd.collective_compute(
                "AllGather", ALU.bypass, ins=[ag_in.opt()], outs=[ag_out.opt()],
                replica_groups=[list(range(NCORES))],
            )
            # ag_out rows are PHYSICAL samples b=0..7; rotate so column i maps
            # to sample (k+i)%8 using the host-provided per-core permutation
            # Pk: wTp[f, i] = sum_b ag_out[b, f] * Pk[b, i].
            lg2 = gp.tile([8, 4], f32r, tag="lg2")
            nc.sync.dma_start(lg2[:], ag_out[:])
            psml7 = ps_m.tile([128, 512], f32, tag="mid")
            pwTp = psml7[0:4, 0:8]
            nc.tensor.matmul(pwTp, lg2[:], Pkv)
            wTp = gp.tile([4, 8], f32r, tag="wTp")
            nc.vector.tensor_copy(wTp[:], pwTp)
            psml8 = ps_m.tile([128, 512], f32, tag="mid")
            pwcol = psml8[:, 0:8]
            nc.tensor.matmul(pwcol, E4v, wTp[:])
            wcol8 = gp.tile([128, 8], f32, tag="wcol8")
            nc.vector.tensor_copy(wcol8[:], pwcol)
            for i in range(1, B):
                wpat[i] = wpp.tile([128, 512], bf16, tag=f"wpat{i}", name=f"wpat{i}")
                nc.vector.tensor_scalar(wpat[i][:], maskv, wcol8[:, i:i + 1],
                                        None, ALU.mult)

            # =================== phase A: forward DFT + S ===================
            SrT = [[None] * 8 for _ in range(B)]
            SiT = [[None] * 8 for _ in range(B)]
            uv_eng = 0
            for i in range(B):
                for g2 in range(2):
                    for cc in range(2):
                        c0 = 4 * g2 + 2 * cc
                        p1 = ps_a.tile([128, 512], f32, tag="pa")
                        for q in range(2):
                            nc.tensor.matmul(
                                p1[:, q * 256:(q + 1) * 256],
                                fb[i][:, (c0 + q) * 128:(c0 + q + 1) * 128],
                                R1v)
                        uv = uvp.tile([128, 512], bf16, tag="uv")
                        if uv_eng == 0:
                            nc.scalar.copy(uv[:], p1[:])
                        else:
                            nc.vector.tensor_copy(uv[:], p1[:])
                        uv_eng ^= 1
                        p2 = ps_a.tile([128, 512], f32, tag="pa")
                        for q in range(2):
                            U = uv[:, q * 256:q * 256 + 128]
                            V = uv[:, q * 256 + 128:(q + 1) * 256]
                            base = q * 130
                            nc.tensor.matmul(p2[:, base:base + 65], U, R2aL,
                                             start=True, stop=False)
                            nc.tensor.matmul(p2[:, base:base + 65], V, R2bL,
                                             start=False, stop=True)
                            nc.tensor.matmul(p2[:, base + 65:base + 130], U,
                                             R2aR, start=True, stop=False)
                            nc.tensor.matmul(p2[:, base + 65:base + 130], V,
                                             R2bR, start=False, stop=True)
                        X2 = qp.tile([128, 260], bf16, tag="X2")
                        if uv_eng == 0:
                            nc.scalar.copy(X2[:], p2[:, 0:260])
                        else:
                            nc.vector.tensor_copy(X2[:], p2[:, 0:260])
                        X2v = X2[:].rearrange("p (q x) -> p q x", q=2)
                        tA = X2v[:, :, 0:65]
                        tB = X2v[:, :, 65:130]
                        uu = qp.tile([128, 130], bf16, tag="uu")
                        vv = qp.tile([128, 130], bf16, tag="vv")
                        uuv = uu[:].rearrange("p (q x) -> p q x", q=2)
                        vvv = vv[:].rearrange("p (q x) -> p q x", q=2)
                        nc.gpsimd.tensor_add(uuv, tA, tB)       # 2*Xr
                        nc.gpsimd.tensor_sub(vvv, tA, tB)       # 2*Xi
                        Sr = srp.tile([128, 130], bf16, tag="Sr")
                        Srv = Sr[:].rearrange("p (q x) -> p q x", q=2)
                        nc.vector.tensor_mul(Srv, tA, tB)       # Xr^2-Xi^2
                        Si = srp.tile([128, 130], bf16, tag="Si")
                        nc.vector.scalar_tensor_tensor(Si[:], uu[:], 0.5,
                                                       vv[:], ALU.mult,
                                                       ALU.mult)  # 2*Xr*Xi
                        SrT[i][2 * g2 + cc] = Sr
                        SiT[i][2 * g2 + cc] = Si

            # =================== phase B: Wmix, product, iDFT ===============
            z2_eng = 0
            res_eng = 0
            for i in range(B):
                for g2 in range(2):
                    pB = ps_o.tile([128, 512], f32, tag="pab")
                    for cc in range(2):
                        c0 = 4 * g2 + 2 * cc
                        pm = ps_m.tile([128, 512], f32, tag="mid")
                        pmv = pm[:, 0:260]
                        for J in range(4):
                            nc.tensor.matmul(
                                pmv, wpat[i][:, J * 128:(J + 1) * 128],
                                ftl[J][:, c0 * 130:(c0 + 2) * 130],
                                start=(J == 0), stop=(J == 3))
                        wmsb = mp.tile([128, 260], bf16, tag="wmsb")
                        if cc == 0:
                            nc.scalar.copy(wmsb[:], pmv)
                        else:
                            nc.vector.tensor_copy(wmsb[:], pmv)
                        wm = wmsb[:].rearrange("p (q x) -> p q x", q=2)
                        wmr = wm[:, :, 0:65]
                        wmi = wm[:, :, 65:130]
                        Sr = SrT[i][2 * g2 + cc]
                        Si = SiT[i][2 * g2 + cc]
                        Srv = Sr[:].rearrange("p (q x) -> p q x", q=2)
                        Siv = Si[:].rearrange("p (q x) -> p q x", q=2)
                        m1 = mp.tile([128, 130], bf16, tag="m1")
                        m2 = mp.tile([128, 130], bf16, tag="m2")
                        m3 = mp.tile([128, 130], bf16, tag="m3")
                        m4 = mp.tile([128, 130], bf16, tag="m4")
                        m1v = m1[:].rearrange("p (q x) -> p q x", q=2)
                        m2v = m2[:].rearrange("p (q x) -> p q x", q=2)
                        m3v = m3[:].rearrange("p (q x) -> p q x", q=2)
                        m4v = m4[:].rearrange("p (q x) -> p q x", q=2)
                        nc.vector.tensor_mul(m1v, Srv, wmr)
                        nc.vector.tensor_mul(m2v, Siv, wmi)
                        nc.gpsimd.tensor_mul(m3v, Srv, wmi)
                        nc.gpsimd.tensor_mul(m4v, Siv, wmr)
                        Dr = drp.tile([128, 130], f32r, tag="Dr")
                        Di = drp.tile([128, 130], f32r, tag="Di")
                        nc.vector.tensor_sub(Dr[:], m1[:], m2[:])
                        nc.gpsimd.tensor_add(Di[:], m3[:], m4[:])
                        pAt = ps_o.tile([128, 512], f32, tag="pab")
                        pA = pAt[0:65, :]
                        for q in range(2):
                            nc.tensor.matmul(pA[:, q * 256:(q + 1) * 256],
                                             Dr[:, q * 65:(q + 1) * 65],
                                             RA1v, start=True, stop=False)
                            nc.tensor.matmul(pA[:, q * 256:(q + 1) * 256],
                                             Di[:, q * 65:(q + 1) * 65],
                                             RA2v, start=False, stop=True)
                        z2 = z2p.tile([65, 512], bf16, tag="z2")
                        if z2_eng == 0:
                            nc.scalar.copy(z2[:], pA)
                        else:
                            nc.vector.tensor_copy(z2[:], pA)
                        z2_eng ^= 1
                        for q in range(2):
                            col = (2 * cc + q) * 128
                            nc.tensor.matmul(pB[:, col:col + 128],
                                             z2[:, q * 256:q * 256 + 128],
                                             RB1v, start=True, stop=False)
                            nc.tensor.matmul(pB[:, col:col + 128],
                                             z2[:, q * 256 + 128:(q + 1) * 256],
                                             RB2v, start=False, stop=True)
                    ot = otp.tile([128, 512], f32, tag="ot")
                    if res_eng == 0:
                        nc.vector.scalar_tensor_tensor(
                            ot[:], fb[i][:, g2 * 512:(g2 + 1) * 512].bitcast(f32),
                            rcolv[:, 0:1], pB[:], ALU.mult, ALU.add)
                    else:
                        # residual via r*I matmul accumulated into pB, then
                        # a plain psum->sbuf copy on the act engine
                        nc.tensor.matmul(pB[:], RIv,
                                         fb[i][:, g2 * 512:(g2 + 1) * 512],
                                         start=False, stop=True)
                        nc.scalar.copy(ot[:], pB[:])
                    res_eng ^= 1
                    nc.sync.dma_start(
                        out_d[i, :, g2 * 512:(g2 + 1) * 512], ot[:])
    nc.compile()
    return nc


def _get_kernel():
    if "nc" not in _cache:
        _cache["nc"] = _build_kernel()
        _cache["consts"] = _build_constants()
    return _cache["nc"], _cache["consts"]


def kernel(**inputs):
    nc, consts = _get_kernel()
    Wblk, bp_col, WgT, wgb, Wt, rw = _prep_params(inputs)
    feat = np.asarray(inputs["features"], np.float32)
    bf = ml_dtypes.bfloat16

    packA = np.zeros((128, NA), np.float32)
    packA[:, A_R1:A_R1 + 256] = consts["R1"]
    packA[:, A_RA1:A_RA1 + 256] = consts["RA1"]
    packA[:, A_RA2:A_RA2 + 256] = consts["RA2"]
    packA[:, A_WBLK:A_WBLK + 128] = Wblk
    packA[:, A_F2:A_F2 + 64] = consts["F2"]
    packA[:, A_G16:A_G16 + 16] = consts["G16"]
    packA[:, A_BP] = bp_col
    packA[0:16, A_E16:A_E16 + 128] = consts["E16"]
    packA[0:4, A_E4:A_E4 + 128] = consts["E4"]
    packA[0:64, A_WGT:A_WGT + 4] = WgT
    packA[0, A_WGB:A_WGB + 4] = wgb
    packA[:, A_RCOL] = rw
    packA[:, A_RI:A_RI + 128] = rw * np.eye(128, dtype=np.float32)

    packB = np.zeros((128, NB), np.float32)
    packB[:, B_R2A:B_R2A + 130] = consts["R2ap"]
    packB[:, B_R2B:B_R2B + 130] = consts["R2bp"]
    packB[:, B_MASK:B_MASK + 512] = consts["maskcat"]
    packB[0:65, B_RB1:B_RB1 + 128] = consts["RB1"]
    packB[0:65, B_RB2:B_RB2 + 128] = consts["RB2"]
    packB = packB.astype(bf)

    in_maps = []
    for k in range(NCORES):
        sl = slice(k * CS, (k + 1) * CS)
        bs = [(k + i) % 8 for i in range(B)]
        pkA = packA.copy()
        for i in range(B):
            pkA[bs[i], A_PK + i] = 1.0
        # featf: [h, (i, c, w)] with block i = physical sample bs[i]
        ff = feat[bs][:, sl].transpose(2, 0, 1, 3).reshape(128, 8192)
        ff = np.ascontiguousarray(ff, np.float32)
        # featg: [(t, c), s] for sample k
        fg = feat[k].reshape(C, 2, 64 * 128).transpose(1, 0, 2) \
                    .reshape(128, 64 * 128)
        fg = np.ascontiguousarray(fg).astype(bf)
        # ftiles: [J, (f, p32), (c, ri, k2)]
        Wts = Wt[:, sl]                                   # [F, CS, H, WF]
        ftiles = np.empty((4, 128, CS * 2 * WF), np.float32)
        for J in range(4):
            blk = Wts[:, :, 32 * J:32 * J + 32, :]        # [F, CS, 32, WF]
            re = blk.real.astype(np.float32)
            im = blk.imag.astype(np.float32)
            stacked = np.stack([re, im], axis=3)          # [F, CS, 32, 2, WF]
            ftiles[J] = stacked.transpose(0, 2, 1, 3, 4).reshape(128, CS * 2 * WF)
        m = {
            "packA": pkA, "packB": packB,
            "featg": fg, "featf": ff,
            "ftl": ftiles.astype(bf),
        }
        in_maps.append(m)

    res = run_bass_kernel_spmd(nc, in_maps, list(range(NCORES)))
    out = np.empty((B, C, H, W), np.float32)
    for k in range(NCORES):
        sl = slice(k * CS, (k + 1) * CS)
        bs = [(k + i) % 8 for i in range(B)]
        o = np.asarray(res.results[k]["out"], np.float32)  # [8, 128, 1024]
        o = o.reshape(8, 128, CS, W).transpose(0, 2, 1, 3)  # [i, c, h, w]
        for i, b in enumerate(bs):
            out[b, sl] = o[i]
    return out


if __name__ == "__main__":
    pass
